# revision 5
# baseline (speedup 1.0000x reference)
"""Trainium2 Bass kernel for CropConLoss (supervised-contrastive style loss).

Contract: kernel(**inputs) takes the FULL unsharded inputs
(protos [64,128] f32, proj2/proj3 [4096,128] f32, target2/target3 [4096] i64)
and returns the FULL output (scalar f32 mean loss), running the compute on
8 NeuronCores via bass_utils.run_bass_kernel_spmd.

Strategy (data-parallel over the M=8192 rows of feats):
  - Host pre-normalizes feats and protos (f32 -> bf16) and rolls each
    core's copy of the 8192 keys so its own 1024 queries come first
    (SPMD-uniform diagonal masking).
  - One-hot class matrices are generated ON DEVICE from a small label
    tensor (iota + is_equal on the vector engine) -- saves 2MB of DMA,
    which was packet-rate bound and gated the main loop.
  - Device per core: 64 key tiles; sim = keysT_kt^T @ qnT (PE, bf16),
    exp via one ACT instruction per tile (constant scale 1/tau), diagonal
    masked by multiplying a [128,128] (1-I) tile for the first 8 tiles,
    per-class + row sums accumulated by one-hot matmuls into a persistent
    PSUM accumulator [65, 1024].
  - Device returns 4 rows (numer_region, rowsum, numer_proto, denom_proto);
    host applies frequency weights, logs, and the mean.
"""

import sys
import types

sys.path.insert(0, "/opt/trn_rl_repo")

import numpy as np

TAU = 0.1
EPS_FREQ = 1e-06
EPS_DENOM = 1e-12

N_CORES = 8
M = 8192          # total rows (2*4096)
D = 128           # feature dim
C = 64            # num classes
Q = M // N_CORES  # 1024 query rows per core
NT = M // 128     # 64 key tiles of 128
KT_CHUNKS = [8, 16, 40]   # key tiles per kt dma chunk (2/4/10KB lines)


def _install_ntff_hook():
    """Shim antenv.axon_hooks (absent in this image) so trace=True works."""
    if "antenv.axon_hooks" in sys.modules:
        return
    try:
        if "/root/.axon_site" not in sys.path:
            sys.path.insert(0, "/root/.axon_site")
        import trn_agent_boot.trn_boot as tb

        hook = tb._ntff_profile_via_ctypes("/opt/axon/libaxon_pjrt.so")
        mod = types.ModuleType("antenv.axon_hooks")
        mod._hook = hook
        mod.get_axon_ntff_profile_hook = lambda: mod._hook
        mod.set_axon_ntff_profile_hook = lambda h: setattr(mod, "_hook", h)
        sys.modules["antenv.axon_hooks"] = mod
        import antenv

        antenv.axon_hooks = mod
    except Exception:
        pass


def build_nc():
    """Build and compile the single-core Bass program (same NEFF on all 8)."""
    import concourse.bass as bass  # noqa: F401
    import concourse.mybir as mybir
    import concourse.bacc as bacc
    from concourse import tile

    f32 = mybir.dt.float32
    bf16 = mybir.dt.bfloat16
    mult = mybir.AluOpType.mult
    Act = mybir.ActivationFunctionType

    nc = bacc.Bacc("TRN2", target_bir_lowering=False, debug=False,
                   num_devices=N_CORES)

    NC_ = len(KT_CHUNKS)
    d_kt = [nc.dram_tensor(f"kt{g}", [128, n * 128], bf16,
                           kind="ExternalInput")
            for g, n in enumerate(KT_CHUNKS)]
    d_protosT = nc.dram_tensor("protosT", [128, C + 1], bf16,
                               kind="ExternalInput")
    d_invdiag = nc.dram_tensor("invdiag", [128, 128], bf16,
                               kind="ExternalInput")
    d_iota = nc.dram_tensor("iotaT", [128, C], f32, kind="ExternalInput")
    d_lab = nc.dram_tensor("labT", [128, NT], f32, kind="ExternalInput")
    d_ohqT = nc.dram_tensor("ohqT", [C + 1, Q], f32, kind="ExternalInput")
    d_cfinv = nc.dram_tensor("cfinv", [C + 1, 1], f32, kind="ExternalInput")
    d_ones = nc.dram_tensor("ones65", [C + 1, 1], bf16, kind="ExternalInput")
    d_onesf = nc.dram_tensor("ones65f", [C + 1, 1], f32,
                             kind="ExternalInput")
    d_out = nc.dram_tensor("out", [4, Q], f32, kind="ExternalOutput")

    with tile.TileContext(nc) as tc:
        with (
            tc.tile_pool(name="const", bufs=1) as cst,
            tc.tile_pool(name="work", bufs=3) as work,
        ):
            # ---- resident SBUF tensors ----
            kt = [cst.tile([128, n * 128], bf16, tag=f"kt{g}", name=f"kt{g}")
                  for g, n in enumerate(KT_CHUNKS)]
            protosT = cst.tile([128, C + 1], bf16, tag="protosT")
            invdiag = cst.tile([128, 128], bf16, tag="invdiag")
            iotaT = cst.tile([128, C], f32, tag="iotaT")
            labT = cst.tile([128, NT], f32, tag="labT")
            ohqT = cst.tile([C + 1, Q], f32, tag="ohqT")
            cfinv = cst.tile([C + 1, 1], f32, tag="cfinv")
            ones65 = cst.tile([C + 1, 1], bf16, tag="ones65")
            ones65f = cst.tile([C + 1, 1], f32, tag="ones65f")
            ohbuf = cst.tile([128, NT, C + 1], bf16, tag="ohbuf")
            p_t = cst.tile([C + 1, Q], f32, tag="p_t")
            tmp = cst.tile([C + 1, Q], bf16, tag="tmp")
            pdrow = cst.tile([1, Q], f32, tag="pdrow")
            nprow = cst.tile([1, Q], f32, tag="nprow")
            pnrow = cst.tile([1, Q], f32, tag="pnrow")
            s0row = cst.tile([1, Q], f32, tag="s0row")
            b2 = cst.tile([C + 1, Q], bf16, tag="b2")

            # warmup: kick off the ACT table load before any data lands
            wu = cst.tile([1, 1], f32, tag="wu")
            nc.vector.memset(wu[:], 0.0)
            wu2 = cst.tile([1, 1], f32, tag="wu2")
            nc.scalar.activation(wu2[:], wu[:], Act.Exp)

            # head DMAs in consumption order
            nc.sync.dma_start(protosT[:], d_protosT[:])
            nc.sync.dma_start(kt[0][:], d_kt[0][:])
            nc.sync.dma_start(iotaT[:], d_iota[:])
            nc.sync.dma_start(labT[:], d_lab[:])
            nc.sync.dma_start(invdiag[:], d_invdiag[:])
            nc.sync.dma_start(ohqT[:], d_ohqT[:])
            nc.sync.dma_start(cfinv[:], d_cfinv[:])
            nc.sync.dma_start(ones65[:], d_ones[:])
            nc.sync.dma_start(ones65f[:], d_onesf[:])
            for g in range(1, NC_):
                nc.sync.dma_start(kt[g][:], d_kt[g][:])

            # device-side one-hot generation: ohbuf[:, t, 1+c] = (lab == c+1)
            for t in range(NT):
                nc.vector.tensor_scalar(
                    ohbuf[:, t, 1:C + 1], iotaT[:], labT[:, t:t + 1], None,
                    op0=mybir.AluOpType.is_equal)
                nc.vector.memset(ohbuf[:, t, 0:1], 1.0)

            def kslice(t):
                if t < KT_CHUNKS[0]:
                    g, s = 0, t
                elif t < KT_CHUNKS[0] + KT_CHUNKS[1]:
                    g, s = 1, t - KT_CHUNKS[0]
                else:
                    g, s = 2, t - KT_CHUNKS[0] - KT_CHUNKS[1]
                return kt[g][:, s * 128:(s + 1) * 128]

            with tc.tile_pool(name="acc", bufs=1, space="PSUM") as acc:
                sT = acc.tile([C + 1, Q], f32, tag="sT")
                with tc.tile_pool(name="ring", bufs=3, space="PSUM") as ring:
                    # ---- proto phase (uses ring slots; qnT == kt[0]) ----
                    pp = ring.tile([128, Q], f32, tag="ps", name="pp")
                    for j in range(Q // 512):
                        nc.tensor.matmul(pp[0:C + 1, j * 512:(j + 1) * 512],
                                         protosT[:],
                                         kt[0][:, j * 512:(j + 1) * 512],
                                         start=True, stop=True)
                    nc.scalar.activation(p_t[:], pp[0:C + 1, :], Act.Exp,
                                         scale=1.0 / TAU)
                    # numer_proto = sum_c ohq[c,q] * p_t[c,q]
                    nc.vector.tensor_tensor(tmp[:], p_t[:], ohqT[:], op=mult)
                    pd = ring.tile([128, Q], f32, tag="ps", name="pd")
                    for j in range(Q // 512):
                        nc.tensor.matmul(pd[0:1, j * 512:(j + 1) * 512],
                                         cfinv[:],
                                         p_t[:, j * 512:(j + 1) * 512],
                                         start=True, stop=True)
                    nc.vector.tensor_copy(pdrow[:], pd[0:1, :])
                    np_ = ring.tile([128, Q], f32, tag="ps", name="np_")
                    for j in range(Q // 512):
                        nc.tensor.matmul(np_[0:1, j * 512:(j + 1) * 512],
                                         ones65[:],
                                         tmp[:, j * 512:(j + 1) * 512],
                                         start=True, stop=True)
                    nc.scalar.copy(nprow[:], np_[0:1, :])

                    # ---- main loop over 64 key tiles ----
                    exp_tiles = {}
                    for t in range(NT):
                        ps = ring.tile([128, Q], f32, tag="ps")
                        for j in range(Q // 512):
                            nc.tensor.matmul(
                                ps[:, j * 512:(j + 1) * 512],
                                kslice(t),
                                kt[0][:, j * 512:(j + 1) * 512],
                                start=True, stop=True)
                        # software-pipelined: class-sum matmul for t-1
                        if t > 0:
                            et_p = exp_tiles.pop(t - 1)
                            for j in range(Q // 512):
                                nc.tensor.matmul(
                                    sT[:, j * 512:(j + 1) * 512],
                                    ohbuf[:, t - 1, :],
                                    et_p[:, j * 512:(j + 1) * 512],
                                    start=(t - 1 == 0), stop=False)
                        et = work.tile([128, Q], bf16, tag="et")
                        nc.scalar.activation(et[:], ps[:], Act.Exp,
                                             scale=1.0 / TAU)
                        if t < 8:
                            nc.vector.tensor_tensor(
                                et[:, t * 128:(t + 1) * 128],
                                et[:, t * 128:(t + 1) * 128],
                                invdiag[:], op=mult)
                        exp_tiles[t] = et
                    et_p = exp_tiles.pop(NT - 1)
                    for j in range(Q // 512):
                        nc.tensor.matmul(
                            sT[:, j * 512:(j + 1) * 512],
                            ohbuf[:, NT - 1, :],
                            et_p[:, j * 512:(j + 1) * 512],
                            start=False, stop=True)

                    # ---- epilogue: 4 rows out; host does logs/weights ----
                    nc.vector.tensor_tensor(b2[:], sT[:], ohqT[:], op=mult)
                    pn = ring.tile([128, Q], f32, tag="ps", name="pn")
                    for j in range(Q // 512):
                        nc.tensor.matmul(pn[0:1, j * 512:(j + 1) * 512],
                                         ones65[:],
                                         b2[:, j * 512:(j + 1) * 512],
                                         start=True, stop=True)
                    nc.scalar.copy(s0row[:], sT[0:1, :])
                    nc.vector.tensor_copy(pnrow[:], pn[0:1, :])
                    nc.sync.dma_start(d_out[0:1, :], pnrow[:])
                    nc.sync.dma_start(d_out[1:2, :], s0row[:])
                    nc.sync.dma_start(d_out[2:3, :], nprow[:])
                    nc.sync.dma_start(d_out[3:4, :], pdrow[:])

    nc.compile()
    return nc


def make_in_maps(protos, proj2, target2, proj3, target3):
    import ml_dtypes

    bf16 = ml_dtypes.bfloat16
    f32 = np.float32

    feats = np.concatenate([np.asarray(proj2, dtype=f32),
                            np.asarray(proj3, dtype=f32)], axis=0)
    labels = np.concatenate([np.asarray(target2), np.asarray(target3)],
                            axis=0).astype(np.int64)

    # host-side normalization (matches reference _l2norm in f32)
    nrm = np.sqrt(np.sum(feats * feats, axis=1, keepdims=True, dtype=f32))
    featsn = (feats / np.maximum(nrm, f32(1e-12))).astype(f32)
    pr = np.asarray(protos, dtype=f32)
    pnrm = np.sqrt(np.sum(pr * pr, axis=1, keepdims=True, dtype=f32))
    prn = (pr / np.maximum(pnrm, f32(1e-12))).astype(f32)

    counts = np.bincount(labels, minlength=C).astype(f32)
    cls_freq = (counts + f32(1.0)) + f32(EPS_FREQ)   # matches reference
    cfr = (f32(1.0) / cls_freq).astype(f32)

    # globals (identical on every core)
    invdiag = (np.ones((128, 128)) - np.eye(128)).astype(bf16)
    cfinv = np.zeros((C + 1, 1), dtype=f32)
    cfinv[1:, 0] = cfr
    ones65 = np.ones((C + 1, 1), dtype=bf16)
    ones65f = np.ones((C + 1, 1), dtype=f32)
    protosT = np.zeros((128, C + 1), dtype=bf16)
    protosT[:, 1:] = np.ascontiguousarray(prn.T).astype(bf16)
    iotaT = np.broadcast_to(np.arange(1, C + 1, dtype=f32), (128, C)).copy()

    in_maps = []
    fw_list = []
    for c in range(N_CORES):
        idx = (np.arange(M) + c * Q) % M
        kf = featsn[idx]                     # [8192, 128] rolled, normalized
        kl = labels[idx]

        keysT = np.ascontiguousarray(kf.T).astype(bf16)   # [128, 8192]
        labT = np.ascontiguousarray(
            (1.0 + kl.reshape(NT, 128).T).astype(f32))    # [128, 64]

        ohqT = np.zeros((C + 1, Q), dtype=f32)
        ohqT[1 + kl[:Q], np.arange(Q)] = f32(1.0)

        fw_list.append(cfr[kl[:Q]].astype(np.float64))

        im = {
            "protosT": protosT,
            "invdiag": invdiag,
            "iotaT": iotaT,
            "labT": labT,
            "ohqT": ohqT,
            "cfinv": cfinv,
            "ones65": ones65,
            "ones65f": ones65f,
        }
        o = 0
        for g, n in enumerate(KT_CHUNKS):
            im[f"kt{g}"] = np.ascontiguousarray(keysT[:, o:o + n * 128])
            o += n * 128
        in_maps.append(im)
    return in_maps, fw_list


def run(in_maps, trace=False):
    _install_ntff_hook()
    from concourse import bass_utils

    nc = build_nc()
    res = bass_utils.run_bass_kernel_spmd(
        nc, in_maps, core_ids=list(range(N_CORES)), trace=trace)
    return res


def _finish(res, fw_list):
    """Host-side epilogue: weights, logs, mean over all cores' rows."""
    tot = np.float64(0.0)
    for i in range(N_CORES):
        o = np.asarray(res.results[i]["out"], dtype=np.float64)
        numer = o[0] + o[2]
        den = o[1] * fw_list[i] + o[3] + EPS_DENOM
        tot += np.sum(np.log(den) - np.log(numer))
    return np.asarray(np.float32(tot / M), dtype=np.float32)


def kernel(protos, proj2, target2, proj3, target3):
    in_maps, fw_list = make_in_maps(protos, proj2, target2, proj3, target3)
    res = run(in_maps, trace=False)
    return _finish(res, fw_list)


# revision 8
# speedup vs baseline: 1.0213x; 1.0213x over previous
"""Trainium2 Bass kernel for CropConLoss (supervised-contrastive style loss).

Contract: kernel(**inputs) takes the FULL unsharded inputs
(protos [64,128] f32, proj2/proj3 [4096,128] f32, target2/target3 [4096] i64)
and returns the FULL output (scalar f32 mean loss), running the compute on
8 NeuronCores via bass_utils.run_bass_kernel_spmd.

Strategy (data-parallel over the M=8192 rows of feats):
  - Host pre-normalizes feats and protos (f32 -> bf16) and rolls each
    core's copy of the 8192 keys so its own 1024 queries come first
    (SPMD-uniform diagonal masking).
  - One-hot class matrices are generated ON DEVICE from a small label
    tensor (iota + is_equal on the vector engine) -- saves 2MB of DMA,
    which was packet-rate bound and gated the main loop.
  - Device per core: 64 key tiles; sim = keysT_kt^T @ qnT (PE, bf16),
    exp via one ACT instruction per tile (constant scale 1/tau), diagonal
    masked by multiplying a [128,128] (1-I) tile for the first 8 tiles,
    per-class + row sums accumulated by one-hot matmuls into a persistent
    PSUM accumulator [65, 1024].
  - Device returns 4 rows (numer_region, rowsum, numer_proto, denom_proto);
    host applies frequency weights, logs, and the mean.
"""

import sys
import types

sys.path.insert(0, "/opt/trn_rl_repo")

import numpy as np

TAU = 0.1
EPS_FREQ = 1e-06
EPS_DENOM = 1e-12

N_CORES = 8
M = 8192          # total rows (2*4096)
D = 128           # feature dim
C = 64            # num classes
Q = M // N_CORES  # 1024 query rows per core
NT = M // 128     # 64 key tiles of 128
KT_CHUNKS = [8, 16, 40]   # key tiles per kt dma chunk (2/4/10KB lines)


def _install_ntff_hook():
    """Shim antenv.axon_hooks (absent in this image) so trace=True works."""
    if "antenv.axon_hooks" in sys.modules:
        return
    try:
        if "/root/.axon_site" not in sys.path:
            sys.path.insert(0, "/root/.axon_site")
        import trn_agent_boot.trn_boot as tb

        hook = tb._ntff_profile_via_ctypes("/opt/axon/libaxon_pjrt.so")
        mod = types.ModuleType("antenv.axon_hooks")
        mod._hook = hook
        mod.get_axon_ntff_profile_hook = lambda: mod._hook
        mod.set_axon_ntff_profile_hook = lambda h: setattr(mod, "_hook", h)
        sys.modules["antenv.axon_hooks"] = mod
        import antenv

        antenv.axon_hooks = mod
    except Exception:
        pass


def build_nc():
    """Build and compile the single-core Bass program (same NEFF on all 8)."""
    import concourse.bass as bass  # noqa: F401
    import concourse.mybir as mybir
    import concourse.bacc as bacc
    from concourse import tile

    f32 = mybir.dt.float32
    bf16 = mybir.dt.bfloat16
    mult = mybir.AluOpType.mult
    Act = mybir.ActivationFunctionType

    nc = bacc.Bacc("TRN2", target_bir_lowering=False, debug=False,
                   num_devices=N_CORES)

    NC_ = len(KT_CHUNKS)
    d_kt = [nc.dram_tensor(f"kt{g}", [128, n * 128], bf16,
                           kind="ExternalInput")
            for g, n in enumerate(KT_CHUNKS)]
    d_protosT = nc.dram_tensor("protosT", [128, C + 1], bf16,
                               kind="ExternalInput")
    d_invdiag = nc.dram_tensor("invdiag", [128, 128], bf16,
                               kind="ExternalInput")
    d_iota = nc.dram_tensor("iotaT", [128, C], f32, kind="ExternalInput")
    d_lab = nc.dram_tensor("labT", [128, NT], f32, kind="ExternalInput")
    d_ohqT = nc.dram_tensor("ohqT", [C + 1, Q], f32, kind="ExternalInput")
    d_cfinv = nc.dram_tensor("cfinv", [C + 1, 1], f32, kind="ExternalInput")
    d_ones = nc.dram_tensor("ones65", [C + 1, 1], bf16, kind="ExternalInput")
    d_onesf = nc.dram_tensor("ones65f", [C + 1, 1], f32,
                             kind="ExternalInput")
    d_out = nc.dram_tensor("out", [4, Q], f32, kind="ExternalOutput")

    with tile.TileContext(nc) as tc:
        with (
            tc.tile_pool(name="const", bufs=1) as cst,
            tc.tile_pool(name="work", bufs=3) as work,
        ):
            # ---- resident SBUF tensors ----
            kt = [cst.tile([128, n * 128], bf16, tag=f"kt{g}", name=f"kt{g}")
                  for g, n in enumerate(KT_CHUNKS)]
            protosT = cst.tile([128, C + 1], bf16, tag="protosT")
            invdiag = cst.tile([128, 128], bf16, tag="invdiag")
            iotaT = cst.tile([128, C], f32, tag="iotaT")
            labT = cst.tile([128, NT], f32, tag="labT")
            ohqT = cst.tile([C + 1, Q], f32, tag="ohqT")
            cfinv = cst.tile([C + 1, 1], f32, tag="cfinv")
            ones65 = cst.tile([C + 1, 1], bf16, tag="ones65")
            ones65f = cst.tile([C + 1, 1], f32, tag="ones65f")
            ohbuf = [cst.tile([128, 8, C + 1], bf16, tag=f"ohbuf{g}",
                              name=f"ohbuf{g}")
                     for g in range(NT // 8)]
            p_t = cst.tile([C + 1, Q], f32, tag="p_t")
            tmp = cst.tile([C + 1, Q], bf16, tag="tmp")
            pdrow = cst.tile([1, Q], f32, tag="pdrow")
            nprow = cst.tile([1, Q], f32, tag="nprow")
            pnrow = cst.tile([1, Q], f32, tag="pnrow")
            s0row = cst.tile([1, Q], f32, tag="s0row")
            b2 = cst.tile([C + 1, Q], bf16, tag="b2")

            # warmup: kick off the ACT table load before any data lands
            wu = cst.tile([1, 1], f32, tag="wu")
            nc.vector.memset(wu[:], 0.0)
            wu2 = cst.tile([1, 1], f32, tag="wu2")
            nc.scalar.activation(wu2[:], wu[:], Act.Exp)

            # head DMAs in consumption order
            nc.sync.dma_start(protosT[:], d_protosT[:])
            nc.sync.dma_start(kt[0][:], d_kt[0][:])
            nc.sync.dma_start(iotaT[:], d_iota[:])
            nc.sync.dma_start(labT[:], d_lab[:])
            nc.sync.dma_start(invdiag[:], d_invdiag[:])
            nc.sync.dma_start(ohqT[:], d_ohqT[:])
            nc.sync.dma_start(cfinv[:], d_cfinv[:])
            nc.sync.dma_start(ones65[:], d_ones[:])
            nc.sync.dma_start(ones65f[:], d_onesf[:])
            for g in range(1, NC_):
                nc.sync.dma_start(kt[g][:], d_kt[g][:])

            # device-side one-hot generation: ohbuf[g][:, s, 1+c] = (lab==c+1)
            # (emitted in chunks of 8 tiles; chunk g+1 is generated while the
            # main loop is consuming chunk g, to keep the DVE FIFO flowing)
            def gen_oh_chunk(g):
                for s in range(8):
                    t = g * 8 + s
                    nc.vector.tensor_scalar(
                        ohbuf[g][:, s, 1:C + 1], iotaT[:],
                        labT[:, t:t + 1], None,
                        op0=mybir.AluOpType.is_equal)
                    nc.vector.memset(ohbuf[g][:, s, 0:1], 1.0)

            def ohslice(t):
                return ohbuf[t // 8][:, t % 8, :]

            def kslice(t):
                if t < KT_CHUNKS[0]:
                    g, s = 0, t
                elif t < KT_CHUNKS[0] + KT_CHUNKS[1]:
                    g, s = 1, t - KT_CHUNKS[0]
                else:
                    g, s = 2, t - KT_CHUNKS[0] - KT_CHUNKS[1]
                return kt[g][:, s * 128:(s + 1) * 128]

            # super-tiles: PSUM ring tiles hold SW=3 512-col chunks, so one
            # ACT instruction covers 1.5 key tiles (amortizes the ~352-cycle
            # fixed ACTIVATE cost). Global 512-col chunk c <-> key tile c//2,
            # query half c%2; super-tile s holds chunks 3s..3s+2.
            NCH = 2 * NT            # 128 chunks
            NS = (NCH + 2) // 3     # 43 super-tiles (last has 2 chunks)

            def chunks_of(s):
                return range(3 * s, min(3 * s + 3, NCH))

            with tc.tile_pool(name="acc", bufs=1, space="PSUM") as acc:
                sT = acc.tile([C + 1, Q], f32, tag="sT")
                with tc.tile_pool(name="ring", bufs=2, space="PSUM") as ring:
                    # ---- proto phase (uses ring slots; qnT == kt[0]) ----
                    pp = ring.tile([128, 1536], f32, tag="ps", name="pp")
                    for j in range(Q // 512):
                        nc.tensor.matmul(pp[0:C + 1, j * 512:(j + 1) * 512],
                                         protosT[:],
                                         kt[0][:, j * 512:(j + 1) * 512],
                                         start=True, stop=True)
                    nc.scalar.activation(p_t[:], pp[0:C + 1, 0:Q], Act.Exp,
                                         scale=1.0 / TAU)
                    # numer_proto = sum_c ohq[c,q] * p_t[c,q]
                    nc.vector.tensor_tensor(tmp[:], p_t[:], ohqT[:], op=mult)
                    pd = ring.tile([128, 1536], f32, tag="ps", name="pd")
                    for j in range(Q // 512):
                        nc.tensor.matmul(pd[0:1, j * 512:(j + 1) * 512],
                                         cfinv[:],
                                         p_t[:, j * 512:(j + 1) * 512],
                                         start=True, stop=True)
                    nc.vector.tensor_copy(pdrow[:], pd[0:1, 0:Q])
                    np_ = ring.tile([128, 1536], f32, tag="ps", name="np_")
                    for j in range(Q // 512):
                        nc.tensor.matmul(np_[0:1, j * 512:(j + 1) * 512],
                                         ones65[:],
                                         tmp[:, j * 512:(j + 1) * 512],
                                         start=True, stop=True)
                    nc.scalar.copy(nprow[:], np_[0:1, 0:Q])
                    nc.sync.dma_start(d_out[2:3, :], nprow[:])
                    nc.sync.dma_start(d_out[3:4, :], pdrow[:])

                    gen_oh_chunk(0)
                    gen_oh_chunk(1)
                    next_gen = 2

                    # ---- main loop over 43 super-tiles ----
                    exp_tiles = {}
                    for s in range(NS):
                        w = 512 * len(list(chunks_of(s)))
                        ps = ring.tile([128, 1536], f32, tag="ps")
                        for c in chunks_of(s):
                            t, h, slot = c // 2, c % 2, c % 3
                            nc.tensor.matmul(
                                ps[:, slot * 512:(slot + 1) * 512],
                                kslice(t),
                                kt[0][:, h * 512:(h + 1) * 512],
                                start=True, stop=True)
                        # software-pipelined: class-sum matmuls for s-1
                        if s > 0:
                            et_p = exp_tiles.pop(s - 1)
                            for c in chunks_of(s - 1):
                                t, h, slot = c // 2, c % 2, c % 3
                                nc.tensor.matmul(
                                    sT[:, h * 512:(h + 1) * 512],
                                    ohslice(t),
                                    et_p[:, slot * 512:(slot + 1) * 512],
                                    start=(t == 0), stop=(t == NT - 1))
                        et = work.tile([128, 1536], bf16, tag="et")
                        nc.scalar.activation(et[:, 0:w], ps[:, 0:w], Act.Exp,
                                             scale=1.0 / TAU)
                        # diagonal mask: key tile t<8's self-block lives in
                        # chunk 2t + t//4 at in-chunk offset (128t % 512)
                        for t in range(8):
                            cm = 2 * t + t // 4
                            if cm // 3 == s:
                                off = (cm % 3) * 512 + (128 * t) % 512
                                nc.vector.tensor_tensor(
                                    et[:, off:off + 128],
                                    et[:, off:off + 128],
                                    invdiag[:], op=mult)
                        exp_tiles[s] = et
                        # keep one-hot generation ~a chunk ahead of use
                        need = min(7, (3 * s + 2) // 2 // 8 + 1)
                        while next_gen <= need:
                            gen_oh_chunk(next_gen)
                            next_gen += 1
                    et_p = exp_tiles.pop(NS - 1)
                    for c in chunks_of(NS - 1):
                        t, h, slot = c // 2, c % 2, c % 3
                        nc.tensor.matmul(
                            sT[:, h * 512:(h + 1) * 512],
                            ohslice(t),
                            et_p[:, slot * 512:(slot + 1) * 512],
                            start=(t == 0), stop=(t == NT - 1))

                    # ---- epilogue: 4 rows out; host does logs/weights ----
                    nc.vector.tensor_tensor(b2[:], sT[:], ohqT[:], op=mult)
                    pn = ring.tile([128, 1536], f32, tag="ps", name="pn")
                    for j in range(Q // 512):
                        nc.tensor.matmul(pn[0:1, j * 512:(j + 1) * 512],
                                         ones65[:],
                                         b2[:, j * 512:(j + 1) * 512],
                                         start=True, stop=True)
                    nc.scalar.copy(s0row[:], sT[0:1, :])
                    nc.vector.tensor_copy(pnrow[:], pn[0:1, 0:Q])
                    nc.sync.dma_start(d_out[0:1, :], pnrow[:])
                    nc.sync.dma_start(d_out[1:2, :], s0row[:])

    nc.compile()
    return nc


def make_in_maps(protos, proj2, target2, proj3, target3):
    import ml_dtypes

    bf16 = ml_dtypes.bfloat16
    f32 = np.float32

    feats = np.concatenate([np.asarray(proj2, dtype=f32),
                            np.asarray(proj3, dtype=f32)], axis=0)
    labels = np.concatenate([np.asarray(target2), np.asarray(target3)],
                            axis=0).astype(np.int64)

    # host-side normalization (matches reference _l2norm in f32)
    nrm = np.sqrt(np.sum(feats * feats, axis=1, keepdims=True, dtype=f32))
    featsn = (feats / np.maximum(nrm, f32(1e-12))).astype(f32)
    pr = np.asarray(protos, dtype=f32)
    pnrm = np.sqrt(np.sum(pr * pr, axis=1, keepdims=True, dtype=f32))
    prn = (pr / np.maximum(pnrm, f32(1e-12))).astype(f32)

    counts = np.bincount(labels, minlength=C).astype(f32)
    cls_freq = (counts + f32(1.0)) + f32(EPS_FREQ)   # matches reference
    cfr = (f32(1.0) / cls_freq).astype(f32)

    # globals (identical on every core)
    invdiag = (np.ones((128, 128)) - np.eye(128)).astype(bf16)
    cfinv = np.zeros((C + 1, 1), dtype=f32)
    cfinv[1:, 0] = cfr
    ones65 = np.ones((C + 1, 1), dtype=bf16)
    ones65f = np.ones((C + 1, 1), dtype=f32)
    protosT = np.zeros((128, C + 1), dtype=bf16)
    protosT[:, 1:] = np.ascontiguousarray(prn.T).astype(bf16)
    iotaT = np.broadcast_to(np.arange(1, C + 1, dtype=f32), (128, C)).copy()

    in_maps = []
    fw_list = []
    for c in range(N_CORES):
        idx = (np.arange(M) + c * Q) % M
        kf = featsn[idx]                     # [8192, 128] rolled, normalized
        kl = labels[idx]

        keysT = np.ascontiguousarray(kf.T).astype(bf16)   # [128, 8192]
        labT = np.ascontiguousarray(
            (1.0 + kl.reshape(NT, 128).T).astype(f32))    # [128, 64]

        ohqT = np.zeros((C + 1, Q), dtype=f32)
        ohqT[1 + kl[:Q], np.arange(Q)] = f32(1.0)

        fw_list.append(cfr[kl[:Q]].astype(np.float64))

        im = {
            "protosT": protosT,
            "invdiag": invdiag,
            "iotaT": iotaT,
            "labT": labT,
            "ohqT": ohqT,
            "cfinv": cfinv,
            "ones65": ones65,
            "ones65f": ones65f,
        }
        o = 0
        for g, n in enumerate(KT_CHUNKS):
            im[f"kt{g}"] = np.ascontiguousarray(keysT[:, o:o + n * 128])
            o += n * 128
        in_maps.append(im)
    return in_maps, fw_list


def run(in_maps, trace=False):
    _install_ntff_hook()
    from concourse import bass_utils

    nc = build_nc()
    res = bass_utils.run_bass_kernel_spmd(
        nc, in_maps, core_ids=list(range(N_CORES)), trace=trace)
    return res


def _finish(res, fw_list):
    """Host-side epilogue: weights, logs, mean over all cores' rows."""
    tot = np.float64(0.0)
    for i in range(N_CORES):
        o = np.asarray(res.results[i]["out"], dtype=np.float64)
        numer = o[0] + o[2]
        den = o[1] * fw_list[i] + o[3] + EPS_DENOM
        tot += np.sum(np.log(den) - np.log(numer))
    return np.asarray(np.float32(tot / M), dtype=np.float32)


def kernel(protos, proj2, target2, proj3, target3):
    in_maps, fw_list = make_in_maps(protos, proj2, target2, proj3, target3)
    res = run(in_maps, trace=False)
    return _finish(res, fw_list)


# revision 11
# speedup vs baseline: 1.0328x; 1.0113x over previous
"""Trainium2 Bass kernel for CropConLoss (supervised-contrastive style loss).

Contract: kernel(**inputs) takes the FULL unsharded inputs
(protos [64,128] f32, proj2/proj3 [4096,128] f32, target2/target3 [4096] i64)
and returns the FULL output (scalar f32 mean loss), running the compute on
8 NeuronCores via bass_utils.run_bass_kernel_spmd.

Strategy (data-parallel over the M=8192 rows of feats):
  - Host pre-normalizes feats and protos (f32 -> bf16) and rolls each
    core's copy of the 8192 keys so its own 1024 queries come first
    (SPMD-uniform diagonal masking).
  - One-hot class matrices are generated ON DEVICE from a small label
    tensor (iota + is_equal on the vector engine) -- saves 2MB of DMA,
    which was packet-rate bound and gated the main loop.
  - Device per core: 64 key tiles; sim = keysT_kt^T @ qnT (PE, bf16),
    exp via one ACT instruction per tile (constant scale 1/tau), diagonal
    masked by multiplying a [128,128] (1-I) tile for the first 8 tiles,
    per-class + row sums accumulated by one-hot matmuls into a persistent
    PSUM accumulator [65, 1024].
  - Device returns 4 rows (numer_region, rowsum, numer_proto, denom_proto);
    host applies frequency weights, logs, and the mean.
"""

import sys
import types

sys.path.insert(0, "/opt/trn_rl_repo")

import numpy as np

TAU = 0.1
EPS_FREQ = 1e-06
EPS_DENOM = 1e-12

N_CORES = 8
M = 8192          # total rows (2*4096)
D = 128           # feature dim
C = 64            # num classes
Q = M // N_CORES  # 1024 query rows per core
NT = M // 128     # 64 key tiles of 128
KT_CHUNKS = [8, 16, 40]   # key tiles per kt dma chunk (2/4/10KB lines)


def _install_ntff_hook():
    """Shim antenv.axon_hooks (absent in this image) so trace=True works."""
    if "antenv.axon_hooks" in sys.modules:
        return
    try:
        if "/root/.axon_site" not in sys.path:
            sys.path.insert(0, "/root/.axon_site")
        import trn_agent_boot.trn_boot as tb

        hook = tb._ntff_profile_via_ctypes("/opt/axon/libaxon_pjrt.so")
        mod = types.ModuleType("antenv.axon_hooks")
        mod._hook = hook
        mod.get_axon_ntff_profile_hook = lambda: mod._hook
        mod.set_axon_ntff_profile_hook = lambda h: setattr(mod, "_hook", h)
        sys.modules["antenv.axon_hooks"] = mod
        import antenv

        antenv.axon_hooks = mod
    except Exception:
        pass


def build_nc():
    """Build and compile the single-core Bass program (same NEFF on all 8)."""
    import concourse.bass as bass  # noqa: F401
    import concourse.mybir as mybir
    import concourse.bacc as bacc
    from concourse import tile

    f32 = mybir.dt.float32
    bf16 = mybir.dt.bfloat16
    mult = mybir.AluOpType.mult
    Act = mybir.ActivationFunctionType

    nc = bacc.Bacc("TRN2", target_bir_lowering=False, debug=False,
                   num_devices=N_CORES)

    NC_ = len(KT_CHUNKS)
    d_kt = [nc.dram_tensor(f"kt{g}", [128, n * 128], bf16,
                           kind="ExternalInput")
            for g, n in enumerate(KT_CHUNKS)]
    d_protosT = nc.dram_tensor("protosT", [128, C + 1], bf16,
                               kind="ExternalInput")
    d_invdiag = nc.dram_tensor("invdiag", [128, 128], bf16,
                               kind="ExternalInput")
    d_iota = nc.dram_tensor("iotaT", [128, C], f32, kind="ExternalInput")
    d_lab = nc.dram_tensor("labT", [128, NT], f32, kind="ExternalInput")
    d_ohqT = nc.dram_tensor("ohqT", [C + 1, Q], f32, kind="ExternalInput")
    d_cfinv = nc.dram_tensor("cfinv", [C + 1, 1], f32, kind="ExternalInput")
    d_ones = nc.dram_tensor("ones65", [C + 1, 1], bf16, kind="ExternalInput")
    d_onesf = nc.dram_tensor("ones65f", [C + 1, 1], f32,
                             kind="ExternalInput")
    d_out = nc.dram_tensor("out", [4, Q], f32, kind="ExternalOutput")

    with tile.TileContext(nc) as tc:
        with (
            tc.tile_pool(name="const", bufs=1) as cst,
            tc.tile_pool(name="work", bufs=3) as work,
        ):
            # ---- resident SBUF tensors ----
            kt = [cst.tile([128, n * 128], bf16, tag=f"kt{g}", name=f"kt{g}")
                  for g, n in enumerate(KT_CHUNKS)]
            protosT = cst.tile([128, C + 1], bf16, tag="protosT")
            invdiag = cst.tile([128, 128], bf16, tag="invdiag")
            iotaT = cst.tile([128, C], f32, tag="iotaT")
            labT = cst.tile([128, NT], f32, tag="labT")
            ohqT = cst.tile([C + 1, Q], f32, tag="ohqT")
            cfinv = cst.tile([C + 1, 1], f32, tag="cfinv")
            ones65 = cst.tile([C + 1, 1], bf16, tag="ones65")
            ones65f = cst.tile([C + 1, 1], f32, tag="ones65f")
            ohbuf = [cst.tile([128, 8, C + 1], bf16, tag=f"ohbuf{g}",
                              name=f"ohbuf{g}")
                     for g in range(NT // 8)]
            p_t = cst.tile([C + 1, Q], f32, tag="p_t")
            tmp = cst.tile([C + 1, Q], bf16, tag="tmp")
            pdrow = cst.tile([1, Q], f32, tag="pdrow")
            nprow = cst.tile([1, Q], f32, tag="nprow")
            pnrow = cst.tile([1, Q], f32, tag="pnrow")
            s0row = cst.tile([1, Q], f32, tag="s0row")
            b2 = cst.tile([C + 1, Q], bf16, tag="b2")

            # warmup: kick off the ACT table load before any data lands
            wu = cst.tile([1, 1], f32, tag="wu")
            nc.vector.memset(wu[:], 0.0)
            wu2 = cst.tile([1, 1], f32, tag="wu2")
            nc.scalar.activation(wu2[:], wu[:], Act.Exp)

            # head DMAs in consumption order
            nc.sync.dma_start(protosT[:], d_protosT[:])
            nc.sync.dma_start(kt[0][:], d_kt[0][:])
            nc.sync.dma_start(iotaT[:], d_iota[:])
            nc.sync.dma_start(labT[:], d_lab[:])
            nc.sync.dma_start(invdiag[:], d_invdiag[:])
            nc.sync.dma_start(ohqT[:], d_ohqT[:])
            nc.sync.dma_start(cfinv[:], d_cfinv[:])
            nc.sync.dma_start(ones65[:], d_ones[:])
            nc.sync.dma_start(ones65f[:], d_onesf[:])
            for g in range(1, NC_):
                nc.sync.dma_start(kt[g][:], d_kt[g][:])

            # device-side one-hot generation: ohbuf[g][:, s, 1+c] = (lab==c+1)
            # (emitted in chunks of 8 tiles; chunk g+1 is generated while the
            # main loop is consuming chunk g, to keep the DVE FIFO flowing)
            def gen_oh_chunk(g):
                for s in range(8):
                    t = g * 8 + s
                    nc.vector.tensor_scalar(
                        ohbuf[g][:, s, 1:C + 1], iotaT[:],
                        labT[:, t:t + 1], None,
                        op0=mybir.AluOpType.is_equal)
                    nc.vector.memset(ohbuf[g][:, s, 0:1], 1.0)

            def ohslice(t):
                return ohbuf[t // 8][:, t % 8, :]

            def kslice(t):
                if t < KT_CHUNKS[0]:
                    g, s = 0, t
                elif t < KT_CHUNKS[0] + KT_CHUNKS[1]:
                    g, s = 1, t - KT_CHUNKS[0]
                else:
                    g, s = 2, t - KT_CHUNKS[0] - KT_CHUNKS[1]
                return kt[g][:, s * 128:(s + 1) * 128]

            # super-tiles: PSUM ring tiles hold SW=3 512-col chunks, so one
            # ACT instruction covers 1.5 key tiles (amortizes the ~352-cycle
            # fixed ACTIVATE cost). Global 512-col chunk c <-> key tile c//2,
            # query half c%2; super-tile s holds chunks 3s..3s+2.
            NCH = 2 * NT            # 128 chunks
            NS = (NCH + 2) // 3     # 43 super-tiles (last has 2 chunks)

            def chunks_of(s):
                return range(3 * s, min(3 * s + 3, NCH))

            with tc.tile_pool(name="acc", bufs=1, space="PSUM") as acc:
                sT = acc.tile([C + 1, Q], f32, tag="sT")
                with tc.tile_pool(name="ring", bufs=2, space="PSUM") as ring:
                    gen_oh_chunk(0)
                    gen_oh_chunk(1)
                    next_gen = 2

                    # ---- proto head (uses ring slots; qnT == kt[0]) ----
                    pp = ring.tile([128, 1536], f32, tag="ps", name="pp")
                    for j in range(Q // 512):
                        nc.tensor.matmul(pp[0:C + 1, j * 512:(j + 1) * 512],
                                         protosT[:],
                                         kt[0][:, j * 512:(j + 1) * 512],
                                         start=True, stop=True)
                    nc.scalar.activation(p_t[:], pp[0:C + 1, 0:Q], Act.Exp,
                                         scale=1.0 / TAU)

                    exp_tiles = {}

                    def sim_exp(s):
                        w = 512 * len(list(chunks_of(s)))
                        ps = ring.tile([128, 1536], f32, tag="ps")
                        for c in chunks_of(s):
                            t, h, slot = c // 2, c % 2, c % 3
                            nc.tensor.matmul(
                                ps[:, slot * 512:(slot + 1) * 512],
                                kslice(t),
                                kt[0][:, h * 512:(h + 1) * 512],
                                start=True, stop=True)
                        et = work.tile([128, 1536], bf16, tag="et")
                        nc.scalar.activation(et[:, 0:w], ps[:, 0:w], Act.Exp,
                                             scale=1.0 / TAU)
                        # diagonal mask: key tile t<8's self-block lives in
                        # chunk 2t + t//4 at in-chunk offset (128t % 512)
                        for t in range(8):
                            cm = 2 * t + t // 4
                            if cm // 3 == s:
                                off = (cm % 3) * 512 + (128 * t) % 512
                                nc.vector.tensor_tensor(
                                    et[:, off:off + 128],
                                    et[:, off:off + 128],
                                    invdiag[:], op=mult)
                        exp_tiles[s] = et

                    def class_mms(s):
                        et_p = exp_tiles.pop(s)
                        for c in chunks_of(s):
                            t, h, slot = c // 2, c % 2, c % 3
                            nc.tensor.matmul(
                                sT[:, h * 512:(h + 1) * 512],
                                ohslice(t),
                                et_p[:, slot * 512:(slot + 1) * 512],
                                start=(t == 0), stop=(t == NT - 1))

                    # prelude: super-tiles 0,1 keep ACT busy while the
                    # proto tail (pd/np_ matmuls + DVE copies) runs off
                    # the exp critical path
                    sim_exp(0)
                    sim_exp(1)
                    # numer_proto = sum_c ohq[c,q] * p_t[c,q]
                    nc.vector.tensor_tensor(tmp[:], p_t[:], ohqT[:], op=mult)
                    pd = ring.tile([128, 1536], f32, tag="ps", name="pd")
                    for j in range(Q // 512):
                        nc.tensor.matmul(pd[0:1, j * 512:(j + 1) * 512],
                                         cfinv[:],
                                         p_t[:, j * 512:(j + 1) * 512],
                                         start=True, stop=True)
                    nc.vector.tensor_copy(pdrow[:], pd[0:1, 0:Q])
                    np_ = ring.tile([128, 1536], f32, tag="ps", name="np_")
                    for j in range(Q // 512):
                        nc.tensor.matmul(np_[0:1, j * 512:(j + 1) * 512],
                                         ones65[:],
                                         tmp[:, j * 512:(j + 1) * 512],
                                         start=True, stop=True)
                    nc.vector.tensor_copy(nprow[:], np_[0:1, 0:Q])
                    nc.sync.dma_start(d_out[2:3, :], nprow[:])
                    nc.sync.dma_start(d_out[3:4, :], pdrow[:])
                    class_mms(0)

                    # ---- main loop over the remaining super-tiles ----
                    for s in range(2, NS):
                        sim_exp(s)
                        class_mms(s - 1)
                        # keep one-hot generation ~a chunk ahead of use
                        need = min(7, (3 * s + 2) // 2 // 8 + 1)
                        while next_gen <= need:
                            gen_oh_chunk(next_gen)
                            next_gen += 1
                    et_p = exp_tiles.pop(NS - 1)
                    for c in chunks_of(NS - 1):
                        t, h, slot = c // 2, c % 2, c % 3
                        nc.tensor.matmul(
                            sT[:, h * 512:(h + 1) * 512],
                            ohslice(t),
                            et_p[:, slot * 512:(slot + 1) * 512],
                            start=(t == 0), stop=(t == NT - 1))

                    # ---- epilogue: 4 rows out; host does logs/weights ----
                    nc.vector.tensor_tensor(b2[:], sT[:], ohqT[:], op=mult)
                    pn = ring.tile([128, 1536], f32, tag="ps", name="pn")
                    for j in range(Q // 512):
                        nc.tensor.matmul(pn[0:1, j * 512:(j + 1) * 512],
                                         ones65[:],
                                         b2[:, j * 512:(j + 1) * 512],
                                         start=True, stop=True)
                    nc.scalar.copy(s0row[:], sT[0:1, :])
                    nc.vector.tensor_copy(pnrow[:], pn[0:1, 0:Q])
                    nc.sync.dma_start(d_out[0:1, :], pnrow[:])
                    nc.sync.dma_start(d_out[1:2, :], s0row[:])

    nc.compile()
    return nc


def make_in_maps(protos, proj2, target2, proj3, target3):
    import ml_dtypes

    bf16 = ml_dtypes.bfloat16
    f32 = np.float32

    feats = np.concatenate([np.asarray(proj2, dtype=f32),
                            np.asarray(proj3, dtype=f32)], axis=0)
    labels = np.concatenate([np.asarray(target2), np.asarray(target3)],
                            axis=0).astype(np.int64)

    # host-side normalization (matches reference _l2norm in f32)
    nrm = np.sqrt(np.sum(feats * feats, axis=1, keepdims=True, dtype=f32))
    featsn = (feats / np.maximum(nrm, f32(1e-12))).astype(f32)
    pr = np.asarray(protos, dtype=f32)
    pnrm = np.sqrt(np.sum(pr * pr, axis=1, keepdims=True, dtype=f32))
    prn = (pr / np.maximum(pnrm, f32(1e-12))).astype(f32)

    counts = np.bincount(labels, minlength=C).astype(f32)
    cls_freq = (counts + f32(1.0)) + f32(EPS_FREQ)   # matches reference
    cfr = (f32(1.0) / cls_freq).astype(f32)

    # globals (identical on every core)
    invdiag = (np.ones((128, 128)) - np.eye(128)).astype(bf16)
    cfinv = np.zeros((C + 1, 1), dtype=f32)
    cfinv[1:, 0] = cfr
    ones65 = np.ones((C + 1, 1), dtype=bf16)
    ones65f = np.ones((C + 1, 1), dtype=f32)
    protosT = np.zeros((128, C + 1), dtype=bf16)
    protosT[:, 1:] = np.ascontiguousarray(prn.T).astype(bf16)
    iotaT = np.broadcast_to(np.arange(1, C + 1, dtype=f32), (128, C)).copy()

    in_maps = []
    fw_list = []
    for c in range(N_CORES):
        idx = (np.arange(M) + c * Q) % M
        kf = featsn[idx]                     # [8192, 128] rolled, normalized
        kl = labels[idx]

        keysT = np.ascontiguousarray(kf.T).astype(bf16)   # [128, 8192]
        labT = np.ascontiguousarray(
            (1.0 + kl.reshape(NT, 128).T).astype(f32))    # [128, 64]

        ohqT = np.zeros((C + 1, Q), dtype=f32)
        ohqT[1 + kl[:Q], np.arange(Q)] = f32(1.0)

        fw_list.append(cfr[kl[:Q]].astype(np.float64))

        im = {
            "protosT": protosT,
            "invdiag": invdiag,
            "iotaT": iotaT,
            "labT": labT,
            "ohqT": ohqT,
            "cfinv": cfinv,
            "ones65": ones65,
            "ones65f": ones65f,
        }
        o = 0
        for g, n in enumerate(KT_CHUNKS):
            im[f"kt{g}"] = np.ascontiguousarray(keysT[:, o:o + n * 128])
            o += n * 128
        in_maps.append(im)
    return in_maps, fw_list


def run(in_maps, trace=False):
    _install_ntff_hook()
    from concourse import bass_utils

    nc = build_nc()
    res = bass_utils.run_bass_kernel_spmd(
        nc, in_maps, core_ids=list(range(N_CORES)), trace=trace)
    return res


def _finish(res, fw_list):
    """Host-side epilogue: weights, logs, mean over all cores' rows."""
    tot = np.float64(0.0)
    for i in range(N_CORES):
        o = np.asarray(res.results[i]["out"], dtype=np.float64)
        numer = o[0] + o[2]
        den = o[1] * fw_list[i] + o[3] + EPS_DENOM
        tot += np.sum(np.log(den) - np.log(numer))
    return np.asarray(np.float32(tot / M), dtype=np.float32)


def kernel(protos, proj2, target2, proj3, target3):
    in_maps, fw_list = make_in_maps(protos, proj2, target2, proj3, target3)
    res = run(in_maps, trace=False)
    return _finish(res, fw_list)


# revision 13
# speedup vs baseline: 1.1620x; 1.1250x over previous
"""Trainium2 Bass kernel for CropConLoss (supervised-contrastive style loss).

Contract: kernel(**inputs) takes the FULL unsharded inputs
(protos [64,128] f32, proj2/proj3 [4096,128] f32, target2/target3 [4096] i64)
and returns the FULL output (scalar f32 mean loss), running the compute on
8 NeuronCores via bass_utils.run_bass_kernel_spmd.

Strategy (data-parallel over the M=8192 rows of feats):
  - Host pre-normalizes feats and protos (f32 -> bf16) and rolls each
    core's copy of the 8192 keys so its own 1024 queries come first
    (SPMD-uniform diagonal masking).
  - One-hot class matrices are generated ON DEVICE from a small label
    tensor (iota + is_equal on the vector engine) -- saves 2MB of DMA,
    which was packet-rate bound and gated the main loop.
  - Device per core: 64 key tiles; sim = keysT_kt^T @ qnT (PE, bf16),
    exp via one ACT instruction per tile (constant scale 1/tau), diagonal
    masked by multiplying a [128,128] (1-I) tile for the first 8 tiles,
    per-class + row sums accumulated by one-hot matmuls into a persistent
    PSUM accumulator [65, 1024].
  - Device returns 4 rows (numer_region, rowsum, numer_proto, denom_proto);
    host applies frequency weights, logs, and the mean.
"""

import sys
import types

sys.path.insert(0, "/opt/trn_rl_repo")

import numpy as np

TAU = 0.1
EPS_FREQ = 1e-06
EPS_DENOM = 1e-12

N_CORES = 8
M = 8192          # total rows (2*4096)
D = 128           # feature dim
C = 64            # num classes
Q = M // N_CORES  # 1024 query rows per core
NT = M // 128     # 64 key tiles of 128
KT_CHUNKS = [8, 16, 40]   # key tiles per kt dma chunk (2/4/10KB lines)


def _install_ntff_hook():
    """Shim antenv.axon_hooks (absent in this image) so trace=True works."""
    if "antenv.axon_hooks" in sys.modules:
        return
    try:
        if "/root/.axon_site" not in sys.path:
            sys.path.insert(0, "/root/.axon_site")
        import trn_agent_boot.trn_boot as tb

        hook = tb._ntff_profile_via_ctypes("/opt/axon/libaxon_pjrt.so")
        mod = types.ModuleType("antenv.axon_hooks")
        mod._hook = hook
        mod.get_axon_ntff_profile_hook = lambda: mod._hook
        mod.set_axon_ntff_profile_hook = lambda h: setattr(mod, "_hook", h)
        sys.modules["antenv.axon_hooks"] = mod
        import antenv

        antenv.axon_hooks = mod
    except Exception:
        pass


def build_nc():
    """Build and compile the single-core Bass program (same NEFF on all 8)."""
    import concourse.bass as bass  # noqa: F401
    import concourse.mybir as mybir
    import concourse.bacc as bacc
    from concourse import tile

    f32 = mybir.dt.float32
    bf16 = mybir.dt.bfloat16
    mult = mybir.AluOpType.mult
    Act = mybir.ActivationFunctionType

    nc = bacc.Bacc("TRN2", target_bir_lowering=False, debug=False,
                   num_devices=N_CORES)

    NC_ = len(KT_CHUNKS)
    d_kt = [nc.dram_tensor(f"kt{g}", [128, n * 128], bf16,
                           kind="ExternalInput")
            for g, n in enumerate(KT_CHUNKS)]
    d_protosT = nc.dram_tensor("protosT", [128, C + 1], bf16,
                               kind="ExternalInput")
    d_invdiag = nc.dram_tensor("invdiag", [128, 128], bf16,
                               kind="ExternalInput")
    d_iota = nc.dram_tensor("iotaT", [128, C], f32, kind="ExternalInput")
    d_lab = nc.dram_tensor("labT", [128, NT], f32, kind="ExternalInput")
    d_opt = nc.dram_tensor("out_pt", [C + 1, Q], f32, kind="ExternalOutput")
    d_ost = nc.dram_tensor("out_sT", [C + 1, Q], f32, kind="ExternalOutput")

    with tile.TileContext(nc) as tc:
        with (
            tc.tile_pool(name="const", bufs=1) as cst,
            tc.tile_pool(name="work", bufs=3) as work,
        ):
            # ---- resident SBUF tensors ----
            kt = [cst.tile([128, n * 128], bf16, tag=f"kt{g}", name=f"kt{g}")
                  for g, n in enumerate(KT_CHUNKS)]
            protosT = cst.tile([128, C + 1], bf16, tag="protosT")
            invdiag = cst.tile([128, 128], bf16, tag="invdiag")
            iotaT = cst.tile([128, C], f32, tag="iotaT")
            labT = cst.tile([128, NT], f32, tag="labT")
            ohbuf = [cst.tile([128, 8, C + 1], bf16, tag=f"ohbuf{g}",
                              name=f"ohbuf{g}")
                     for g in range(NT // 8)]
            p_t = cst.tile([C + 1, Q], f32, tag="p_t")
            sTc = cst.tile([C + 1, Q], f32, tag="sTc")

            # warmup: kick off the ACT table load before any data lands
            wu = cst.tile([1, 1], f32, tag="wu")
            nc.vector.memset(wu[:], 0.0)
            wu2 = cst.tile([1, 1], f32, tag="wu2")
            nc.scalar.activation(wu2[:], wu[:], Act.Exp)

            # head DMAs in consumption order
            nc.sync.dma_start(protosT[:], d_protosT[:])
            nc.sync.dma_start(kt[0][:], d_kt[0][:])
            nc.sync.dma_start(iotaT[:], d_iota[:])
            nc.sync.dma_start(labT[:], d_lab[:])
            nc.sync.dma_start(invdiag[:], d_invdiag[:])
            for g in range(1, NC_):
                nc.sync.dma_start(kt[g][:], d_kt[g][:])

            # device-side one-hot generation: ohbuf[g][:, s, 1+c] = (lab==c+1)
            # (emitted in chunks of 8 tiles; chunk g+1 is generated while the
            # main loop is consuming chunk g, to keep the DVE FIFO flowing)
            def gen_oh_chunk(g):
                for s in range(8):
                    t = g * 8 + s
                    nc.vector.tensor_scalar(
                        ohbuf[g][:, s, 1:C + 1], iotaT[:],
                        labT[:, t:t + 1], None,
                        op0=mybir.AluOpType.is_equal)
                    nc.vector.memset(ohbuf[g][:, s, 0:1], 1.0)

            def ohslice(t):
                return ohbuf[t // 8][:, t % 8, :]

            def kslice(t):
                if t < KT_CHUNKS[0]:
                    g, s = 0, t
                elif t < KT_CHUNKS[0] + KT_CHUNKS[1]:
                    g, s = 1, t - KT_CHUNKS[0]
                else:
                    g, s = 2, t - KT_CHUNKS[0] - KT_CHUNKS[1]
                return kt[g][:, s * 128:(s + 1) * 128]

            # super-tiles: PSUM ring tiles hold SW=3 512-col chunks, so one
            # ACT instruction covers 1.5 key tiles (amortizes the ~352-cycle
            # fixed ACTIVATE cost). Global 512-col chunk c <-> key tile c//2,
            # query half c%2; super-tile s holds chunks 3s..3s+2.
            NCH = 2 * NT            # 128 chunks
            NS = (NCH + 2) // 3     # 43 super-tiles (last has 2 chunks)

            def chunks_of(s):
                return range(3 * s, min(3 * s + 3, NCH))

            with tc.tile_pool(name="acc", bufs=1, space="PSUM") as acc:
                sT = acc.tile([C + 1, Q], f32, tag="sT")
                with tc.tile_pool(name="ring", bufs=2, space="PSUM") as ring:
                    gen_oh_chunk(0)
                    gen_oh_chunk(1)
                    next_gen = 2

                    # ---- proto head (uses ring slots; qnT == kt[0]) ----
                    pp = ring.tile([128, 1536], f32, tag="ps", name="pp")
                    for j in range(Q // 512):
                        nc.tensor.matmul(pp[0:C + 1, j * 512:(j + 1) * 512],
                                         protosT[:],
                                         kt[0][:, j * 512:(j + 1) * 512],
                                         start=True, stop=True)
                    nc.scalar.activation(p_t[:], pp[0:C + 1, 0:Q], Act.Exp,
                                         scale=1.0 / TAU)

                    exp_tiles = {}

                    def sim_exp(s):
                        w = 512 * len(list(chunks_of(s)))
                        ps = ring.tile([128, 1536], f32, tag="ps")
                        for c in chunks_of(s):
                            t, h, slot = c // 2, c % 2, c % 3
                            nc.tensor.matmul(
                                ps[:, slot * 512:(slot + 1) * 512],
                                kslice(t),
                                kt[0][:, h * 512:(h + 1) * 512],
                                start=True, stop=True)
                        et = work.tile([128, 1536], bf16, tag="et")
                        nc.scalar.activation(et[:, 0:w], ps[:, 0:w], Act.Exp,
                                             scale=1.0 / TAU)
                        # diagonal mask: key tile t<8's self-block lives in
                        # chunk 2t + t//4 at in-chunk offset (128t % 512)
                        for t in range(8):
                            cm = 2 * t + t // 4
                            if cm // 3 == s:
                                off = (cm % 3) * 512 + (128 * t) % 512
                                nc.vector.tensor_tensor(
                                    et[:, off:off + 128],
                                    et[:, off:off + 128],
                                    invdiag[:], op=mult)
                        exp_tiles[s] = et

                    def class_mms(s):
                        et_p = exp_tiles.pop(s)
                        for c in chunks_of(s):
                            t, h, slot = c // 2, c % 2, c % 3
                            nc.tensor.matmul(
                                sT[:, h * 512:(h + 1) * 512],
                                ohslice(t),
                                et_p[:, slot * 512:(slot + 1) * 512],
                                start=(t == 0), stop=(t == NT - 1))

                    # prelude: 3 super-tiles deep so the ACT stream never
                    # waits on the PE pipeline (2-behind class matmuls)
                    sim_exp(0)
                    sim_exp(1)
                    sim_exp(2)
                    class_mms(0)
                    nc.sync.dma_start(d_opt[:], p_t[:])

                    # ---- main loop over the remaining super-tiles ----
                    for s in range(3, NS):
                        sim_exp(s)
                        class_mms(s - 2)
                        # keep one-hot generation ~a chunk ahead of use
                        need = min(7, (3 * s + 2) // 2 // 8 + 1)
                        while next_gen <= need:
                            gen_oh_chunk(next_gen)
                            next_gen += 1
                    class_mms(NS - 2)
                    class_mms(NS - 1)

                    # ---- epilogue: ship sT + p_t; host does the rest ----
                    nc.scalar.copy(sTc[:], sT[:])
                    nc.sync.dma_start(d_ost[:], sTc[:])

    nc.compile()
    return nc


def make_in_maps(protos, proj2, target2, proj3, target3):
    import ml_dtypes

    bf16 = ml_dtypes.bfloat16
    f32 = np.float32

    feats = np.concatenate([np.asarray(proj2, dtype=f32),
                            np.asarray(proj3, dtype=f32)], axis=0)
    labels = np.concatenate([np.asarray(target2), np.asarray(target3)],
                            axis=0).astype(np.int64)

    # host-side normalization (matches reference _l2norm in f32)
    nrm = np.sqrt(np.sum(feats * feats, axis=1, keepdims=True, dtype=f32))
    featsn = (feats / np.maximum(nrm, f32(1e-12))).astype(f32)
    pr = np.asarray(protos, dtype=f32)
    pnrm = np.sqrt(np.sum(pr * pr, axis=1, keepdims=True, dtype=f32))
    prn = (pr / np.maximum(pnrm, f32(1e-12))).astype(f32)

    counts = np.bincount(labels, minlength=C).astype(f32)
    cls_freq = (counts + f32(1.0)) + f32(EPS_FREQ)   # matches reference
    cfr = (f32(1.0) / cls_freq).astype(f32)

    # globals (identical on every core)
    invdiag = (np.ones((128, 128)) - np.eye(128)).astype(bf16)
    protosT = np.zeros((128, C + 1), dtype=bf16)
    protosT[:, 1:] = np.ascontiguousarray(prn.T).astype(bf16)
    iotaT = np.broadcast_to(np.arange(1, C + 1, dtype=f32), (128, C)).copy()

    in_maps = []
    fw_list = []
    make_in_maps.cfr = cfr.astype(np.float64)
    for c in range(N_CORES):
        idx = (np.arange(M) + c * Q) % M
        kf = featsn[idx]                     # [8192, 128] rolled, normalized
        kl = labels[idx]

        keysT = np.ascontiguousarray(kf.T).astype(bf16)   # [128, 8192]
        labT = np.ascontiguousarray(
            (1.0 + kl.reshape(NT, 128).T).astype(f32))    # [128, 64]

        fw_list.append(kl[:Q].copy())

        im = {
            "protosT": protosT,
            "invdiag": invdiag,
            "iotaT": iotaT,
            "labT": labT,
        }
        o = 0
        for g, n in enumerate(KT_CHUNKS):
            im[f"kt{g}"] = np.ascontiguousarray(keysT[:, o:o + n * 128])
            o += n * 128
        in_maps.append(im)
    return in_maps, fw_list


def run(in_maps, trace=False):
    _install_ntff_hook()
    from concourse import bass_utils

    nc = build_nc()
    res = bass_utils.run_bass_kernel_spmd(
        nc, in_maps, core_ids=list(range(N_CORES)), trace=trace)
    return res


def _finish(res, fw_list):
    """Host-side epilogue: gather, weights, logs, mean over all cores."""
    cfr = make_in_maps.cfr                      # [64] float64
    tot = np.float64(0.0)
    qi = np.arange(Q)
    for i in range(N_CORES):
        pt = np.asarray(res.results[i]["out_pt"], dtype=np.float64)
        sT = np.asarray(res.results[i]["out_sT"], dtype=np.float64)
        kl = fw_list[i]                         # own-query labels [1024]
        numer = sT[1 + kl, qi] + pt[1 + kl, qi]
        den = sT[0] * cfr[kl] + (pt[1:] * cfr[:, None]).sum(axis=0) \
            + EPS_DENOM
        tot += np.sum(np.log(den) - np.log(numer))
    return np.asarray(np.float32(tot / M), dtype=np.float32)


def kernel(protos, proj2, target2, proj3, target3):
    in_maps, fw_list = make_in_maps(protos, proj2, target2, proj3, target3)
    res = run(in_maps, trace=False)
    return _finish(res, fw_list)


# revision 14
# speedup vs baseline: 1.1864x; 1.0210x over previous
"""Trainium2 Bass kernel for CropConLoss (supervised-contrastive style loss).

Contract: kernel(**inputs) takes the FULL unsharded inputs
(protos [64,128] f32, proj2/proj3 [4096,128] f32, target2/target3 [4096] i64)
and returns the FULL output (scalar f32 mean loss), running the compute on
8 NeuronCores via bass_utils.run_bass_kernel_spmd.

Strategy (data-parallel over the M=8192 rows of feats):
  - Host pre-normalizes feats and protos (f32 -> bf16) and rolls each
    core's copy of the 8192 keys so its own 1024 queries come first
    (SPMD-uniform diagonal masking).
  - One-hot class matrices are generated ON DEVICE from a small label
    tensor (iota + is_equal on the vector engine) -- saves 2MB of DMA,
    which was packet-rate bound and gated the main loop.
  - Device per core: 64 key tiles; sim = keysT_kt^T @ qnT (PE, bf16),
    exp via one ACT instruction per tile (constant scale 1/tau), diagonal
    masked by multiplying a [128,128] (1-I) tile for the first 8 tiles,
    per-class + row sums accumulated by one-hot matmuls into a persistent
    PSUM accumulator [65, 1024].
  - Device returns 4 rows (numer_region, rowsum, numer_proto, denom_proto);
    host applies frequency weights, logs, and the mean.
"""

import sys
import types

sys.path.insert(0, "/opt/trn_rl_repo")

import numpy as np

TAU = 0.1
EPS_FREQ = 1e-06
EPS_DENOM = 1e-12

N_CORES = 8
M = 8192          # total rows (2*4096)
D = 128           # feature dim
C = 64            # num classes
Q = M // N_CORES  # 1024 query rows per core
NT = M // 128     # 64 key tiles of 128
KT_CHUNKS = [8, 16, 40]   # key tiles per kt dma chunk (2/4/10KB lines)


def _install_ntff_hook():
    """Shim antenv.axon_hooks (absent in this image) so trace=True works."""
    if "antenv.axon_hooks" in sys.modules:
        return
    try:
        if "/root/.axon_site" not in sys.path:
            sys.path.insert(0, "/root/.axon_site")
        import trn_agent_boot.trn_boot as tb

        hook = tb._ntff_profile_via_ctypes("/opt/axon/libaxon_pjrt.so")
        mod = types.ModuleType("antenv.axon_hooks")
        mod._hook = hook
        mod.get_axon_ntff_profile_hook = lambda: mod._hook
        mod.set_axon_ntff_profile_hook = lambda h: setattr(mod, "_hook", h)
        sys.modules["antenv.axon_hooks"] = mod
        import antenv

        antenv.axon_hooks = mod
    except Exception:
        pass


def build_nc():
    """Build and compile the single-core Bass program (same NEFF on all 8)."""
    import concourse.bass as bass  # noqa: F401
    import concourse.mybir as mybir
    import concourse.bacc as bacc
    from concourse import tile

    f32 = mybir.dt.float32
    bf16 = mybir.dt.bfloat16
    mult = mybir.AluOpType.mult
    Act = mybir.ActivationFunctionType

    nc = bacc.Bacc("TRN2", target_bir_lowering=False, debug=False,
                   num_devices=N_CORES)

    NC_ = len(KT_CHUNKS)
    d_kt = [nc.dram_tensor(f"kt{g}", [128, n * 128], bf16,
                           kind="ExternalInput")
            for g, n in enumerate(KT_CHUNKS)]
    d_aux = nc.dram_tensor("aux", [128, 2 * C + 128 + C + 1], f32,
                           kind="ExternalInput")
    d_opt = nc.dram_tensor("out_pt", [C + 1, Q], bf16, kind="ExternalOutput")
    d_ost = nc.dram_tensor("out_sT", [C + 1, Q], bf16,
                           kind="ExternalOutput")

    with tile.TileContext(nc) as tc:
        with (
            tc.tile_pool(name="const", bufs=1) as cst,
            tc.tile_pool(name="work", bufs=3) as work,
        ):
            # ---- resident SBUF tensors ----
            kt = [cst.tile([128, n * 128], bf16, tag=f"kt{g}", name=f"kt{g}")
                  for g, n in enumerate(KT_CHUNKS)]
            aux = cst.tile([128, 2 * C + 128 + C + 1], f32, tag="aux")
            protosT = cst.tile([128, C + 1], bf16, tag="protosT")
            invdiag = cst.tile([128, 128], bf16, tag="invdiag")
            ohbuf = [cst.tile([128, 8, C + 1], bf16, tag=f"ohbuf{g}",
                              name=f"ohbuf{g}")
                     for g in range(NT // 8)]
            p_t = cst.tile([C + 1, Q], bf16, tag="p_t")
            sTc = cst.tile([C + 1, Q], bf16, tag="sTc")

            # warmup: kick off the ACT table load before any data lands
            wu = cst.tile([1, 1], f32, tag="wu")
            nc.vector.memset(wu[:], 0.0)
            wu2 = cst.tile([1, 1], f32, tag="wu2")
            nc.scalar.activation(wu2[:], wu[:], Act.Exp)

            # head DMAs in consumption order (aux packs iota/lab/invdiag/
            # protos into one wide-line transfer; bf16 views are cast below)
            nc.sync.dma_start(aux[:], d_aux[:])
            nc.sync.dma_start(kt[0][:], d_kt[0][:])
            for g in range(1, NC_):
                nc.sync.dma_start(kt[g][:], d_kt[g][:])
            iotaT = aux[:, 0:C]
            labT = aux[:, C:2 * C]
            nc.vector.tensor_copy(invdiag[:], aux[:, 2 * C:2 * C + 128])
            nc.vector.tensor_copy(protosT[:],
                                  aux[:, 2 * C + 128:2 * C + 128 + C + 1])

            # device-side one-hot generation: ohbuf[g][:, s, 1+c] = (lab==c+1)
            # (emitted in chunks of 8 tiles; chunk g+1 is generated while the
            # main loop is consuming chunk g, to keep the DVE FIFO flowing)
            def gen_oh_chunk(g):
                for s in range(8):
                    t = g * 8 + s
                    nc.vector.tensor_scalar(
                        ohbuf[g][:, s, 1:C + 1], iotaT,
                        labT[:, t:t + 1], None,
                        op0=mybir.AluOpType.is_equal)
                nc.vector.memset(ohbuf[g][:, :, 0:1], 1.0)

            def ohslice(t):
                return ohbuf[t // 8][:, t % 8, :]

            def kslice(t):
                if t < KT_CHUNKS[0]:
                    g, s = 0, t
                elif t < KT_CHUNKS[0] + KT_CHUNKS[1]:
                    g, s = 1, t - KT_CHUNKS[0]
                else:
                    g, s = 2, t - KT_CHUNKS[0] - KT_CHUNKS[1]
                return kt[g][:, s * 128:(s + 1) * 128]

            # super-tiles: PSUM ring tiles hold SW=3 512-col chunks, so one
            # ACT instruction covers 1.5 key tiles (amortizes the ~352-cycle
            # fixed ACTIVATE cost). Global 512-col chunk c <-> key tile c//2,
            # query half c%2; super-tile s holds chunks 3s..3s+2.
            NCH = 2 * NT            # 128 chunks
            NS = (NCH + 2) // 3     # 43 super-tiles (last has 2 chunks)

            def chunks_of(s):
                return range(3 * s, min(3 * s + 3, NCH))

            with tc.tile_pool(name="acc", bufs=1, space="PSUM") as acc:
                sT = acc.tile([C + 1, Q], f32, tag="sT")
                with tc.tile_pool(name="ring", bufs=2, space="PSUM") as ring:
                    gen_oh_chunk(0)
                    gen_oh_chunk(1)
                    next_gen = 2

                    # ---- proto head (uses ring slots; qnT == kt[0]) ----
                    pp = ring.tile([128, 1536], f32, tag="ps", name="pp")
                    for j in range(Q // 512):
                        nc.tensor.matmul(pp[0:C + 1, j * 512:(j + 1) * 512],
                                         protosT[:],
                                         kt[0][:, j * 512:(j + 1) * 512],
                                         start=True, stop=True)
                    nc.scalar.activation(p_t[:], pp[0:C + 1, 0:Q], Act.Exp,
                                         scale=1.0 / TAU)

                    exp_tiles = {}

                    def sim_exp(s):
                        w = 512 * len(list(chunks_of(s)))
                        ps = ring.tile([128, 1536], f32, tag="ps")
                        for c in chunks_of(s):
                            t, h, slot = c // 2, c % 2, c % 3
                            nc.tensor.matmul(
                                ps[:, slot * 512:(slot + 1) * 512],
                                kslice(t),
                                kt[0][:, h * 512:(h + 1) * 512],
                                start=True, stop=True)
                        et = work.tile([128, 1536], bf16, tag="et")
                        nc.scalar.activation(et[:, 0:w], ps[:, 0:w], Act.Exp,
                                             scale=1.0 / TAU)
                        # diagonal mask: key tile t<8's self-block lives in
                        # chunk 2t + t//4 at in-chunk offset (128t % 512)
                        for t in range(8):
                            cm = 2 * t + t // 4
                            if cm // 3 == s:
                                off = (cm % 3) * 512 + (128 * t) % 512
                                nc.vector.tensor_tensor(
                                    et[:, off:off + 128],
                                    et[:, off:off + 128],
                                    invdiag[:], op=mult)
                        exp_tiles[s] = et

                    def class_mms(s):
                        et_p = exp_tiles.pop(s)
                        for c in chunks_of(s):
                            t, h, slot = c // 2, c % 2, c % 3
                            nc.tensor.matmul(
                                sT[:, h * 512:(h + 1) * 512],
                                ohslice(t),
                                et_p[:, slot * 512:(slot + 1) * 512],
                                start=(t == 0), stop=(t == NT - 1))

                    # prelude: 3 super-tiles deep so the ACT stream never
                    # waits on the PE pipeline (2-behind class matmuls)
                    sim_exp(0)
                    sim_exp(1)
                    sim_exp(2)
                    class_mms(0)
                    nc.sync.dma_start(d_opt[:], p_t[:])

                    # ---- main loop over the remaining super-tiles ----
                    for s in range(3, NS):
                        sim_exp(s)
                        class_mms(s - 2)
                        # keep one-hot generation ~a chunk ahead of use
                        need = min(7, (3 * s + 2) // 2 // 8 + 1)
                        while next_gen <= need:
                            gen_oh_chunk(next_gen)
                            next_gen += 1
                    class_mms(NS - 2)
                    class_mms(NS - 1)

                    # ---- epilogue: ship sT + p_t; host does the rest ----
                    nc.scalar.copy(sTc[:], sT[:])
                    nc.sync.dma_start(d_ost[:], sTc[:])

    nc.compile()
    return nc


def make_in_maps(protos, proj2, target2, proj3, target3):
    import ml_dtypes

    bf16 = ml_dtypes.bfloat16
    f32 = np.float32

    feats = np.concatenate([np.asarray(proj2, dtype=f32),
                            np.asarray(proj3, dtype=f32)], axis=0)
    labels = np.concatenate([np.asarray(target2), np.asarray(target3)],
                            axis=0).astype(np.int64)

    # host-side normalization (matches reference _l2norm in f32)
    nrm = np.sqrt(np.sum(feats * feats, axis=1, keepdims=True, dtype=f32))
    featsn = (feats / np.maximum(nrm, f32(1e-12))).astype(f32)
    pr = np.asarray(protos, dtype=f32)
    pnrm = np.sqrt(np.sum(pr * pr, axis=1, keepdims=True, dtype=f32))
    prn = (pr / np.maximum(pnrm, f32(1e-12))).astype(f32)

    counts = np.bincount(labels, minlength=C).astype(f32)
    cls_freq = (counts + f32(1.0)) + f32(EPS_FREQ)   # matches reference
    cfr = (f32(1.0) / cls_freq).astype(f32)

    # globals (identical on every core): aux packs iota|lab|invdiag|protosT
    invdiagf = (np.ones((128, 128)) - np.eye(128)).astype(f32)
    protosTf = np.zeros((128, C + 1), dtype=f32)
    protosTf[:, 1:] = prn.T
    iotaT = np.broadcast_to(np.arange(1, C + 1, dtype=f32), (128, C))

    in_maps = []
    fw_list = []
    make_in_maps.cfr = cfr.astype(np.float64)
    for c in range(N_CORES):
        idx = (np.arange(M) + c * Q) % M
        kf = featsn[idx]                     # [8192, 128] rolled, normalized
        kl = labels[idx]

        keysT = np.ascontiguousarray(kf.T).astype(bf16)   # [128, 8192]
        labT = np.ascontiguousarray(
            (1.0 + kl.reshape(NT, 128).T).astype(f32))    # [128, 64]

        fw_list.append(kl[:Q].copy())

        im = {
            "aux": np.ascontiguousarray(
                np.concatenate([iotaT, labT, invdiagf, protosTf], axis=1)),
        }
        o = 0
        for g, n in enumerate(KT_CHUNKS):
            im[f"kt{g}"] = np.ascontiguousarray(keysT[:, o:o + n * 128])
            o += n * 128
        in_maps.append(im)
    return in_maps, fw_list


def run(in_maps, trace=False):
    _install_ntff_hook()
    from concourse import bass_utils

    nc = build_nc()
    res = bass_utils.run_bass_kernel_spmd(
        nc, in_maps, core_ids=list(range(N_CORES)), trace=trace)
    return res


def _finish(res, fw_list):
    """Host-side epilogue: gather, weights, logs, mean over all cores."""
    cfr = make_in_maps.cfr                      # [64] float64
    tot = np.float64(0.0)
    qi = np.arange(Q)
    for i in range(N_CORES):
        pt = np.asarray(res.results[i]["out_pt"], dtype=np.float64)
        sT = np.asarray(res.results[i]["out_sT"], dtype=np.float64)
        kl = fw_list[i]                         # own-query labels [1024]
        numer = sT[1 + kl, qi] + pt[1 + kl, qi]
        den = sT[0] * cfr[kl] + (pt[1:] * cfr[:, None]).sum(axis=0) \
            + EPS_DENOM
        tot += np.sum(np.log(den) - np.log(numer))
    return np.asarray(np.float32(tot / M), dtype=np.float32)


def kernel(protos, proj2, target2, proj3, target3):
    in_maps, fw_list = make_in_maps(protos, proj2, target2, proj3, target3)
    res = run(in_maps, trace=False)
    return _finish(res, fw_list)
